# revision 9
# baseline (speedup 1.0000x reference)
"""Trainium2 Bass kernel (v12) for nn_AttentionBlock — Gram-matrix reassociated
causal attention.

Reference (per batch b):
    qs[t,j]    = sum_i s[t,i] Q[h,i,j]
    Omega[t,u] = sum_j qs[t,j] s[u,j]       (causal: keep u <= t)
    es[u,i]    = sum_j E[h,i,j] s[u,j]
    r[t,i]     = sum_h sum_u Omega[t,u] es[u,i]

Key reassociation: the off-diagonal (full blocks u < bt*128) part factors
through the HEAD-INDEPENDENT Gram matrix C(bt)[j,j'] = sum_{u<bt*128} s[u,j]s[u,j']:
    r_off[t,:] = qs[t,:] @ C(bt) @ E_h^T
so per (h, bt):  M^T[j',t] = C(bt)[j,j'] qsT[j,t]  (+ diag:  s[u,j'] OmaskT[u,t])
                 rT[i,t]  += E_h^T[j',i] M^T[j',t]
C(bt) is computed ONCE (prefix-accumulated in one PSUM bank, snapshot per
block), cutting matmul FLOPs ~20% vs a per-head es/G chain, with N=512 streams
vectorized over all 8 heads. Only diagonal 128x128 Omega^T blocks are
materialized, masked by one DVE multiply per 4-head half.

Profiling findings baked into v12 (graded time = last-matmul-end - ~6.1us
preamble + ~12.1us fixed framework epilogue, so only last-MM-end matters):
  - inputs pre-swizzled host-side to partition-major (contiguous 0.5-4KB
    per-partition runs) and spread across ALL FIVE engine DGE queues with
    small first-needed chunks, so the first real matmul starts ~4us after
    the preamble instead of ~6.7us and phase 1 never stalls on data;
  - 7 warm-up matmuls on a zeroed scratch tile issued while DMAs land keep
    the PE HAM activity window busy so real matmuls run at 2.4GHz from the
    start (v11 ran its whole first phase at 1.2GHz, HAM flipped 12us in);
  - per-ic single-bank rT PSUM tiles so the final pair's drain + output DMA
    overlap its remaining matmuls.

Distribution: data-parallel over batch (8 batches = 8 cores, no collectives).
All matmuls bf16; f32 PSUM accumulation. Output computed transposed; host
unswizzles.

PSUM: 5 transient 1-bank slots + 1 Gram bank + 2 rT banks = 8 banks.
"""

import numpy as np
import ml_dtypes

import concourse.bacc as bacc
import concourse.mybir as mybir
import concourse.tile as tile
from concourse.bass_utils import run_bass_kernel_spmd

B = 8      # batch (== number of cores)
T = 1024   # tokens
NF = 256   # feature dim n
H = 8      # heads
P = 128    # partitions
TB = T // P    # 8 token blocks
JC = NF // P   # 2 feature chunks
NCORES = 8

F32 = mybir.dt.float32
BF16 = mybir.dt.bfloat16
IS_GE = mybir.AluOpType.is_ge


def _emit(tc, nc, s_d, sT_d, Q_d, ET_d, out_d, ctx):
    res = ctx.enter_context(tc.tile_pool(name="res", bufs=1))
    omdp = ctx.enter_context(tc.tile_pool(name="omdp", bufs=2))
    msp = ctx.enter_context(tc.tile_pool(name="msp", bufs=2))
    pap = ctx.enter_context(tc.tile_pool(name="pap", bufs=5, space="PSUM"))
    pcp = ctx.enter_context(tc.tile_pool(name="pcp", bufs=1, space="PSUM"))
    prp = ctx.enter_context(tc.tile_pool(name="prp", bufs=2, space="PSUM"))

    s_sb = res.tile([P, TB, NF], BF16)        # [u%128, uc, j]
    sT_sb = res.tile([P, 2, JC, 512], BF16)   # [j%128, tcx, jc, t%512]
    Q_sb = res.tile([P, H * JC, NF], BF16)    # [i%128, h*2+ic, j]
    ET_sb = res.tile([P, H * JC, NF], BF16)   # [j'%128, h*2+jpc, i]
    qsT = res.tile([P, JC, H, T], BF16)       # [j%128, jc, h, t]
    csnap = res.tile([P, JC, TB - 1, NF], BF16)  # [j%128, jc, k, j'] = Csum(k+1)
    mask4 = res.tile([P, 4, P], BF16)         # [u, 4, t]: 1 where u <= t
    scr = res.tile([P, 512], BF16)            # warm-up scratch
    r_out = res.tile([P, TB // 2, JC, 2 * P], F32)  # [i%128, pair, ic, t%256]

    # Input DMAs: first-needed-first, deadline-balanced over the three DGE
    # queues (sync/scalar HWDGE ~64B/ns from ~1.7us post-preamble; gpsimd
    # SWDGE ~180B/ns but ~3.5us descriptor-gen lead time).
    nc.sync.dma_start(out=sT_sb[:, 0, 0], in_=sT_d[:, 0, 0])
    nc.scalar.dma_start(out=Q_sb[:, 0:4, :], in_=Q_d[:, 0:4, :])
    nc.sync.dma_start(out=Q_sb[:, 12:, :], in_=Q_d[:, 12:, :])
    nc.scalar.dma_start(out=Q_sb[:, 4:8, :], in_=Q_d[:, 4:8, :])
    nc.sync.dma_start(out=sT_sb[:, 1], in_=sT_d[:, 1])
    nc.scalar.dma_start(out=Q_sb[:, 8:12, :], in_=Q_d[:, 8:12, :])
    nc.gpsimd.dma_start(out=sT_sb[:, 0, 1], in_=sT_d[:, 0, 1])
    nc.gpsimd.dma_start(out=s_sb, in_=s_d)
    nc.gpsimd.memset(mask4, 1.0)
    nc.gpsimd.affine_select(
        out=mask4, in_=mask4,
        pattern=[[0, 4], [1, P]],
        compare_op=IS_GE,   # keep 1.0 where t - u >= 0, else 0
        fill=0.0, base=0, channel_multiplier=-1,
    )
    nc.gpsimd.dma_start(out=ET_sb[:, 0:8, :], in_=ET_d[:, 0:8, :])
    nc.gpsimd.dma_start(out=ET_sb[:, 8:, :], in_=ET_d[:, 8:, :])

    # PE warm-up: HAM un-throttles after ~3.4us of sustained activity; burn
    # the input-DMA head on dummy matmuls so real ones start at 2.4GHz.
    nc.vector.memset(scr, 0.0)
    wu = pap.tile([P, 512], F32, tag="pw", name="wu")
    for w in range(11):
        nc.tensor.matmul(
            wu, lhsT=scr[:, 0:P], rhs=scr,
            start=(w == 0), stop=(w == 10), skip_group_check=True)

    movers = [nc.vector.tensor_copy, nc.scalar.copy]
    mv = [0]

    def mover(out, in_):
        movers[mv[0] % 2](out=out, in_=in_)
        mv[0] += 1

    # ---- phase 1: qsT for all heads + Gram prefix chain ----
    Cp = pcp.tile([P, JC, NF], F32, name="Cp")

    def qs_group(h, jc, tcx):
        pw = pap.tile([P, 512], F32, tag="pw", name="pwq")
        for ic in range(JC):
            nc.tensor.matmul(
                pw,
                lhsT=Q_sb[:, h * JC + ic, jc * P:(jc + 1) * P],
                rhs=sT_sb[:, tcx, ic, :],
                start=(ic == 0), stop=(ic == JC - 1),
                skip_group_check=True,
            )
        mover(qsT[:, jc, h, tcx * 512:(tcx + 1) * 512], pw)

    def c_link(uc):
        # Cp += s[uc]^T s[uc]; snapshot Csum(uc+1) for the bt loop
        for jc in range(JC):
            nc.tensor.matmul(
                Cp[:, jc, :],
                lhsT=s_sb[:, uc, jc * P:(jc + 1) * P],
                rhs=s_sb[:, uc, :],
                start=(uc == 0 and jc == 0),
                stop=(uc == TB - 2 and jc == JC - 1),
                skip_group_check=True,
            )
        mover(csnap[:, :, uc, :], Cp)

    qjobs = [(h, jc, tcx) for tcx in range(2) for h in range(H)
             for jc in range(JC)]
    k = 0
    for gi, (h, jc, tcx) in enumerate(qjobs):
        qs_group(h, jc, tcx)
        if gi >= 11 and gi % 2 == 1 and k < TB - 1:
            c_link(k)
            k += 1

    # ---- phase 2: bt loop ----
    def rt_group(pair, ms, last):
        prs = [prp.tile([P, 2 * P], F32, tag="pr", name=f"pr{ic}")
               for ic in range(JC)]
        for ic in range(JC):
            n = 0
            for h in range(H):
                for jpc in range(JC):
                    nc.tensor.matmul(
                        prs[ic],
                        lhsT=ET_sb[:, h * JC + jpc, ic * P:(ic + 1) * P],
                        rhs=ms[:, jpc, h, :],
                        start=(n == 0), stop=(n == H * JC - 1),
                        skip_group_check=True,
                    )
                    n += 1
            # drain + store this half while the other half's matmuls run
            eng = [nc.vector.tensor_copy, nc.scalar.copy][ic]
            eng(out=r_out[:, pair, ic, :], in_=prs[ic])
            q = [nc.sync, nc.scalar][ic] if last else [nc.sync, nc.scalar][pair % 2]
            q.dma_start(out=out_d[:, pair, ic], in_=r_out[:, pair, ic])

    ms_cur = None
    ms_prev = None
    for bt in range(TB):
        pair, sub = divmod(bt, 2)
        if sub == 0:
            ms_cur = msp.tile([P, JC, H, 2 * P], BF16, tag="ms",
                              name=f"ms{pair}")
        # diagonal OmegaT for all 8 heads: OmT[u, h, t] = sum_j s[u,j] qsT[j,t]
        oms = []
        for half in range(2):
            om = pap.tile([P, 4, P], F32, tag="pw", name=f"om{half}")
            for jc in range(JC):
                nc.tensor.matmul(
                    om,
                    lhsT=sT_sb[:, bt // 4, jc,
                               (bt % 4) * P:(bt % 4 + 1) * P],
                    rhs=qsT[:, jc, 4 * half:4 * half + 4, bt * P:(bt + 1) * P],
                    start=(jc == 0), stop=(jc == JC - 1),
                    skip_group_check=True,
                )
            oms.append(om)
        # off-diagonal via Gram: M^T[j', h, t] = Csum(bt)[j,j'] qsT[j, h, t]
        mps = []
        for jpc in range(JC):
            for half in range(2):
                mp = pap.tile([P, 4, P], F32, tag="pw", name=f"mp{jpc}{half}")
                if bt >= 1:
                    for jc in range(JC):
                        nc.tensor.matmul(
                            mp,
                            lhsT=csnap[:, jc, bt - 1, jpc * P:(jpc + 1) * P],
                            rhs=qsT[:, jc, 4 * half:4 * half + 4,
                                    bt * P:(bt + 1) * P],
                            start=(jc == 0), stop=False,
                            skip_group_check=True,
                        )
                mps.append(mp)
        # previous pair's rT matmuls slot in here: keeps the PE busy while
        # this bt's masks run on the DVE, and gives the msnap movers slack
        if bt >= 2 and sub == 0:
            rt_group(pair - 1, ms_prev, last=False)
        # causal mask on the diagonal blocks (keep u <= t)
        omd = omdp.tile([P, H, P], BF16, tag="omd", name="omd")
        nc.vector.tensor_mul(omd[:, 0:4, :], oms[0], mask4)
        nc.vector.tensor_mul(omd[:, 4:8, :], oms[1], mask4)
        # diag contribution: M^T[j', h, t] += s[u, j'] OmaskT[u, h, t]
        for jpc in range(JC):
            for half in range(2):
                nc.tensor.matmul(
                    mps[jpc * 2 + half],
                    lhsT=s_sb[:, bt, jpc * P:(jpc + 1) * P],
                    rhs=omd[:, 4 * half:4 * half + 4, :],
                    start=(bt == 0), stop=True,
                    skip_group_check=True,
                )
        for jpc in range(JC):
            for half in range(2):
                mover(ms_cur[:, jpc, 4 * half:4 * half + 4,
                             sub * P:(sub + 1) * P],
                      mps[jpc * 2 + half])
        if sub == 1:
            ms_prev = ms_cur
    rt_group(TB // 2 - 1, ms_prev, last=True)


def build():
    from contextlib import ExitStack

    nc = bacc.Bacc(
        "TRN2",
        target_bir_lowering=False,
        debug=False,
        enable_asserts=False,
        num_devices=NCORES,
    )
    s_d = nc.dram_tensor("s", [P, TB, NF], BF16, kind="ExternalInput").ap()
    sT_d = nc.dram_tensor(
        "sT", [P, 2, JC, 512], BF16, kind="ExternalInput").ap()
    Q_d = nc.dram_tensor(
        "Q", [P, H * JC, NF], BF16, kind="ExternalInput").ap()
    ET_d = nc.dram_tensor(
        "ET", [P, H * JC, NF], BF16, kind="ExternalInput").ap()
    out_d = nc.dram_tensor(
        "out", [P, TB // 2, JC, 2 * P], F32, kind="ExternalOutput").ap()
    with tile.TileContext(nc) as tc:
        with ExitStack() as ctx:
            _emit(tc, nc, s_d, sT_d, Q_d, ET_d, out_d, ctx)
    nc.compile()
    return nc


_NC = None


def _get_nc():
    global _NC
    if _NC is None:
        _NC = build()
    return _NC


def _in_maps(s, Q, E):
    bf = ml_dtypes.bfloat16
    s = np.asarray(s, dtype=np.float32)
    Q = np.asarray(Q, dtype=np.float32)
    E = np.asarray(E, dtype=np.float32)
    # partition-major swizzles (one contiguous run per partition per DMA)
    # Q_sw[p, h*2+ic, j] = Q[h, ic*128+p, j]
    Q_sw = np.ascontiguousarray(
        Q.reshape(H, JC, P, NF).transpose(2, 0, 1, 3).reshape(P, H * JC, NF)
    ).astype(bf)
    # ET_sw[p, h*2+jpc, i] = E[h, i, jpc*128+p]
    ET_sw = np.ascontiguousarray(
        E.transpose(0, 2, 1).reshape(H, JC, P, NF).transpose(2, 0, 1, 3)
        .reshape(P, H * JC, NF)).astype(bf)
    maps = []
    for b in range(B):
        sb = s[b]
        # s_sw[p, uc, j] = s[uc*128+p, j]
        s_sw = np.ascontiguousarray(
            sb.reshape(TB, P, NF).transpose(1, 0, 2)).astype(bf)
        # sT_sw[p, tcx, jc, t'] = s[tcx*512+t', jc*128+p]
        sT_sw = np.ascontiguousarray(
            sb.T.reshape(JC, P, 2, 512).transpose(1, 2, 0, 3)).astype(bf)
        maps.append({"s": s_sw, "sT": sT_sw, "Q": Q_sw, "ET": ET_sw})
    return maps


def _unswizzle_out(o):
    # o[p, pair, ic, t'] -> r[pair*256+t', ic*128+p]
    return np.ascontiguousarray(
        o.transpose(1, 3, 2, 0).reshape(T, NF))


def kernel(s, Q, E):
    nc = _get_nc()
    res = run_bass_kernel_spmd(
        nc, _in_maps(s, Q, E), core_ids=list(range(NCORES)))
    return np.stack(
        [_unswizzle_out(res.results[b]["out"]) for b in range(B)], axis=0)


def run_profiled(s, Q, E, tmpdir=None):
    nc = _get_nc()
    res = run_bass_kernel_spmd(
        nc, _in_maps(s, Q, E), core_ids=list(range(NCORES)),
        trace=True, tmpdir=tmpdir)
    out = np.stack(
        [_unswizzle_out(res.results[b]["out"]) for b in range(B)], axis=0)
    return out, res.exec_time_ns


# revision 12
# speedup vs baseline: 1.0050x; 1.0050x over previous
"""Trainium2 Bass kernel (v12) for nn_AttentionBlock — Gram-matrix reassociated
causal attention.

Reference (per batch b):
    qs[t,j]    = sum_i s[t,i] Q[h,i,j]
    Omega[t,u] = sum_j qs[t,j] s[u,j]       (causal: keep u <= t)
    es[u,i]    = sum_j E[h,i,j] s[u,j]
    r[t,i]     = sum_h sum_u Omega[t,u] es[u,i]

Key reassociation: the off-diagonal (full blocks u < bt*128) part factors
through the HEAD-INDEPENDENT Gram matrix C(bt)[j,j'] = sum_{u<bt*128} s[u,j]s[u,j']:
    r_off[t,:] = qs[t,:] @ C(bt) @ E_h^T
so per (h, bt):  M^T[j',t] = C(bt)[j,j'] qsT[j,t]  (+ diag:  s[u,j'] OmaskT[u,t])
                 rT[i,t]  += E_h^T[j',i] M^T[j',t]
C(bt) is computed ONCE (prefix-accumulated in one PSUM bank, snapshot per
block), cutting matmul FLOPs ~20% vs a per-head es/G chain, with N=512 streams
vectorized over all 8 heads. Only diagonal 128x128 Omega^T blocks are
materialized, masked by one DVE multiply per 4-head half.

Profiling findings baked into v12 (graded time = last-matmul-end - ~6.1us
preamble + ~12.1us fixed framework epilogue, so only last-MM-end matters):
  - inputs pre-swizzled host-side to partition-major (contiguous 0.5-4KB
    per-partition runs) and spread across ALL FIVE engine DGE queues with
    small first-needed chunks, so the first real matmul starts ~4us after
    the preamble instead of ~6.7us and phase 1 never stalls on data;
  - 7 warm-up matmuls on a zeroed scratch tile issued while DMAs land keep
    the PE HAM activity window busy so real matmuls run at 2.4GHz from the
    start (v11 ran its whole first phase at 1.2GHz, HAM flipped 12us in);
  - per-ic single-bank rT PSUM tiles so the final pair's drain + output DMA
    overlap its remaining matmuls.

Distribution: data-parallel over batch (8 batches = 8 cores, no collectives).
All matmuls bf16; f32 PSUM accumulation. Output computed transposed; host
unswizzles.

PSUM: 5 transient 1-bank slots + 1 Gram bank + 2 rT banks = 8 banks.
"""

import numpy as np
import ml_dtypes

import concourse.bacc as bacc
import concourse.mybir as mybir
import concourse.tile as tile
from concourse.bass_utils import run_bass_kernel_spmd

B = 8      # batch (== number of cores)
T = 1024   # tokens
NF = 256   # feature dim n
H = 8      # heads
P = 128    # partitions
TB = T // P    # 8 token blocks
JC = NF // P   # 2 feature chunks
NCORES = 8

F32 = mybir.dt.float32
BF16 = mybir.dt.bfloat16
IS_GE = mybir.AluOpType.is_ge


def _emit(tc, nc, s_d, sT_d, Q_d, ET_d, out_d, ctx):
    res = ctx.enter_context(tc.tile_pool(name="res", bufs=1))
    omdp = ctx.enter_context(tc.tile_pool(name="omdp", bufs=2))
    msp = ctx.enter_context(tc.tile_pool(name="msp", bufs=2))
    pap = ctx.enter_context(tc.tile_pool(name="pap", bufs=5, space="PSUM"))
    pcp = ctx.enter_context(tc.tile_pool(name="pcp", bufs=1, space="PSUM"))
    prp = ctx.enter_context(tc.tile_pool(name="prp", bufs=2, space="PSUM"))

    s_sb = res.tile([P, TB, NF], BF16)        # [u%128, uc, j]
    sT_sb = res.tile([P, 2, JC, 512], BF16)   # [j%128, tcx, jc, t%512]
    Q_sb = res.tile([P, H * JC, NF], BF16)    # [i%128, h*2+ic, j]
    ET_sb = res.tile([P, H * JC, NF], BF16)   # [j'%128, h*2+jpc, i]
    qsT = res.tile([P, JC, H, T], BF16)       # [j%128, jc, h, t]
    csnap = res.tile([P, JC, TB - 1, NF], BF16)  # [j%128, jc, k, j'] = Csum(k+1)
    mask4 = res.tile([P, 4, P], BF16)         # [u, 4, t]: 1 where u <= t
    scr = res.tile([P, 512], BF16)            # warm-up scratch
    r_out = res.tile([P, TB // 2, JC, 2 * P], F32)  # [i%128, pair, ic, t%256]

    # Input DMAs: first-needed-first, deadline-balanced over the three DGE
    # queues (sync/scalar HWDGE ~64B/ns from ~1.7us post-preamble; gpsimd
    # SWDGE ~180B/ns but ~3.5us descriptor-gen lead time).
    nc.sync.dma_start(out=sT_sb[:, 0, 0], in_=sT_d[:, 0, 0])
    nc.scalar.dma_start(out=Q_sb[:, 0:4, :], in_=Q_d[:, 0:4, :])
    nc.sync.dma_start(out=Q_sb[:, 12:, :], in_=Q_d[:, 12:, :])
    nc.scalar.dma_start(out=Q_sb[:, 8:12, :], in_=Q_d[:, 8:12, :])
    nc.sync.dma_start(out=sT_sb[:, 1], in_=sT_d[:, 1])
    nc.gpsimd.dma_start(out=sT_sb[:, 0, 1], in_=sT_d[:, 0, 1])
    nc.gpsimd.dma_start(out=Q_sb[:, 4:8, :], in_=Q_d[:, 4:8, :])
    nc.gpsimd.dma_start(out=s_sb, in_=s_d)
    nc.gpsimd.memset(mask4, 1.0)
    nc.gpsimd.affine_select(
        out=mask4, in_=mask4,
        pattern=[[0, 4], [1, P]],
        compare_op=IS_GE,   # keep 1.0 where t - u >= 0, else 0
        fill=0.0, base=0, channel_multiplier=-1,
    )
    nc.gpsimd.dma_start(out=ET_sb[:, 0:8, :], in_=ET_d[:, 0:8, :])
    nc.sync.dma_start(out=ET_sb[:, 8:, :], in_=ET_d[:, 8:, :])

    # PE warm-up: HAM un-throttles after ~3.4us of sustained activity; burn
    # the input-DMA head on dummy matmuls so real ones start at 2.4GHz.
    nc.vector.memset(scr, 0.0)
    wu = pap.tile([P, 512], F32, tag="pw", name="wu")
    for w in range(10):
        nc.tensor.matmul(
            wu, lhsT=scr[:, 0:P], rhs=scr,
            start=(w == 0), stop=(w == 9), skip_group_check=True)

    movers = [nc.vector.tensor_copy, nc.scalar.copy]
    mv = [0]

    def mover(out, in_):
        movers[mv[0] % 2](out=out, in_=in_)
        mv[0] += 1

    # ---- phase 1: qsT for all heads + Gram prefix chain ----
    Cp = pcp.tile([P, JC, NF], F32, name="Cp")

    def qs_group(h, jc, tcx):
        pw = pap.tile([P, 512], F32, tag="pw", name="pwq")
        for ic in range(JC):
            nc.tensor.matmul(
                pw,
                lhsT=Q_sb[:, h * JC + ic, jc * P:(jc + 1) * P],
                rhs=sT_sb[:, tcx, ic, :],
                start=(ic == 0), stop=(ic == JC - 1),
                skip_group_check=True,
            )
        mover(qsT[:, jc, h, tcx * 512:(tcx + 1) * 512], pw)

    def c_link(uc):
        # Cp += s[uc]^T s[uc]; snapshot Csum(uc+1) for the bt loop
        for jc in range(JC):
            nc.tensor.matmul(
                Cp[:, jc, :],
                lhsT=s_sb[:, uc, jc * P:(jc + 1) * P],
                rhs=s_sb[:, uc, :],
                start=(uc == 0 and jc == 0),
                stop=(uc == TB - 2 and jc == JC - 1),
                skip_group_check=True,
            )
        mover(csnap[:, :, uc, :], Cp)

    qjobs = [(h, jc, tcx) for tcx in range(2) for h in range(H)
             for jc in range(JC)]
    k = 0
    for gi, (h, jc, tcx) in enumerate(qjobs):
        qs_group(h, jc, tcx)
        if gi >= 11 and gi % 2 == 1 and k < TB - 1:
            c_link(k)
            k += 1

    # ---- phase 2: bt loop ----
    def rt_group(pair, ms, last):
        prs = [prp.tile([P, 2 * P], F32, tag="pr", name=f"pr{ic}")
               for ic in range(JC)]
        for ic in range(JC):
            n = 0
            for h in range(H):
                for jpc in range(JC):
                    nc.tensor.matmul(
                        prs[ic],
                        lhsT=ET_sb[:, h * JC + jpc, ic * P:(ic + 1) * P],
                        rhs=ms[:, jpc, h, :],
                        start=(n == 0), stop=(n == H * JC - 1),
                        skip_group_check=True,
                    )
                    n += 1
            # drain + store this half while the other half's matmuls run
            eng = [nc.vector.tensor_copy, nc.scalar.copy][ic]
            eng(out=r_out[:, pair, ic, :], in_=prs[ic])
            q = [nc.sync, nc.scalar][ic] if last else [nc.sync, nc.scalar][pair % 2]
            q.dma_start(out=out_d[:, pair, ic], in_=r_out[:, pair, ic])

    ms_cur = None
    ms_prev = None
    for bt in range(TB):
        pair, sub = divmod(bt, 2)
        if sub == 0:
            ms_cur = msp.tile([P, JC, H, 2 * P], BF16, tag="ms",
                              name=f"ms{pair}")
        # diagonal OmegaT for all 8 heads: OmT[u, h, t] = sum_j s[u,j] qsT[j,t]
        oms = []
        for half in range(2):
            om = pap.tile([P, 4, P], F32, tag="pw", name=f"om{half}")
            for jc in range(JC):
                nc.tensor.matmul(
                    om,
                    lhsT=sT_sb[:, bt // 4, jc,
                               (bt % 4) * P:(bt % 4 + 1) * P],
                    rhs=qsT[:, jc, 4 * half:4 * half + 4, bt * P:(bt + 1) * P],
                    start=(jc == 0), stop=(jc == JC - 1),
                    skip_group_check=True,
                )
            oms.append(om)
        # off-diagonal via Gram: M^T[j', h, t] = Csum(bt)[j,j'] qsT[j, h, t]
        mps = []
        for jpc in range(JC):
            for half in range(2):
                mp = pap.tile([P, 4, P], F32, tag="pw", name=f"mp{jpc}{half}")
                if bt >= 1:
                    for jc in range(JC):
                        nc.tensor.matmul(
                            mp,
                            lhsT=csnap[:, jc, bt - 1, jpc * P:(jpc + 1) * P],
                            rhs=qsT[:, jc, 4 * half:4 * half + 4,
                                    bt * P:(bt + 1) * P],
                            start=(jc == 0), stop=False,
                            skip_group_check=True,
                        )
                mps.append(mp)
        # previous pair's rT matmuls slot in here: keeps the PE busy while
        # this bt's masks run on the DVE, and gives the msnap movers slack
        if bt >= 2 and sub == 0:
            rt_group(pair - 1, ms_prev, last=False)
        # causal mask on the diagonal blocks (keep u <= t)
        omd = omdp.tile([P, H, P], BF16, tag="omd", name="omd")
        nc.vector.tensor_mul(omd[:, 0:4, :], oms[0], mask4)
        nc.vector.tensor_mul(omd[:, 4:8, :], oms[1], mask4)
        # diag contribution: M^T[j', h, t] += s[u, j'] OmaskT[u, h, t]
        for jpc in range(JC):
            for half in range(2):
                nc.tensor.matmul(
                    mps[jpc * 2 + half],
                    lhsT=s_sb[:, bt, jpc * P:(jpc + 1) * P],
                    rhs=omd[:, 4 * half:4 * half + 4, :],
                    start=(bt == 0), stop=True,
                    skip_group_check=True,
                )
        # snap engines 3:1 scalar:vector — vector also owns the masks
        snap_engs = [nc.scalar.copy, nc.vector.tensor_copy,
                     nc.scalar.copy, nc.scalar.copy]
        for jpc in range(JC):
            for half in range(2):
                snap_engs[jpc * 2 + half](
                    out=ms_cur[:, jpc, 4 * half:4 * half + 4,
                               sub * P:(sub + 1) * P],
                    in_=mps[jpc * 2 + half])
        if sub == 1:
            ms_prev = ms_cur
    rt_group(TB // 2 - 1, ms_prev, last=True)


def build():
    from contextlib import ExitStack

    nc = bacc.Bacc(
        "TRN2",
        target_bir_lowering=False,
        debug=False,
        enable_asserts=False,
        num_devices=NCORES,
    )
    s_d = nc.dram_tensor("s", [P, TB, NF], BF16, kind="ExternalInput").ap()
    sT_d = nc.dram_tensor(
        "sT", [P, 2, JC, 512], BF16, kind="ExternalInput").ap()
    Q_d = nc.dram_tensor(
        "Q", [P, H * JC, NF], BF16, kind="ExternalInput").ap()
    ET_d = nc.dram_tensor(
        "ET", [P, H * JC, NF], BF16, kind="ExternalInput").ap()
    out_d = nc.dram_tensor(
        "out", [P, TB // 2, JC, 2 * P], F32, kind="ExternalOutput").ap()
    with tile.TileContext(nc) as tc:
        with ExitStack() as ctx:
            _emit(tc, nc, s_d, sT_d, Q_d, ET_d, out_d, ctx)
    nc.compile()
    return nc


_NC = None


def _get_nc():
    global _NC
    if _NC is None:
        _NC = build()
    return _NC


def _in_maps(s, Q, E):
    bf = ml_dtypes.bfloat16
    s = np.asarray(s, dtype=np.float32)
    Q = np.asarray(Q, dtype=np.float32)
    E = np.asarray(E, dtype=np.float32)
    # partition-major swizzles (one contiguous run per partition per DMA)
    # Q_sw[p, h*2+ic, j] = Q[h, ic*128+p, j]
    Q_sw = np.ascontiguousarray(
        Q.reshape(H, JC, P, NF).transpose(2, 0, 1, 3).reshape(P, H * JC, NF)
    ).astype(bf)
    # ET_sw[p, h*2+jpc, i] = E[h, i, jpc*128+p]
    ET_sw = np.ascontiguousarray(
        E.transpose(0, 2, 1).reshape(H, JC, P, NF).transpose(2, 0, 1, 3)
        .reshape(P, H * JC, NF)).astype(bf)
    maps = []
    for b in range(B):
        sb = s[b]
        # s_sw[p, uc, j] = s[uc*128+p, j]
        s_sw = np.ascontiguousarray(
            sb.reshape(TB, P, NF).transpose(1, 0, 2)).astype(bf)
        # sT_sw[p, tcx, jc, t'] = s[tcx*512+t', jc*128+p]
        sT_sw = np.ascontiguousarray(
            sb.T.reshape(JC, P, 2, 512).transpose(1, 2, 0, 3)).astype(bf)
        maps.append({"s": s_sw, "sT": sT_sw, "Q": Q_sw, "ET": ET_sw})
    return maps


def _unswizzle_out(o):
    # o[p, pair, ic, t'] -> r[pair*256+t', ic*128+p]
    return np.ascontiguousarray(
        o.transpose(1, 3, 2, 0).reshape(T, NF))


def kernel(s, Q, E):
    nc = _get_nc()
    res = run_bass_kernel_spmd(
        nc, _in_maps(s, Q, E), core_ids=list(range(NCORES)))
    return np.stack(
        [_unswizzle_out(res.results[b]["out"]) for b in range(B)], axis=0)


def run_profiled(s, Q, E, tmpdir=None):
    nc = _get_nc()
    res = run_bass_kernel_spmd(
        nc, _in_maps(s, Q, E), core_ids=list(range(NCORES)),
        trace=True, tmpdir=tmpdir)
    out = np.stack(
        [_unswizzle_out(res.results[b]["out"]) for b in range(B)], axis=0)
    return out, res.exec_time_ns


# revision 13
# speedup vs baseline: 1.0102x; 1.0052x over previous
"""Trainium2 Bass kernel (v12) for nn_AttentionBlock — Gram-matrix reassociated
causal attention.

Reference (per batch b):
    qs[t,j]    = sum_i s[t,i] Q[h,i,j]
    Omega[t,u] = sum_j qs[t,j] s[u,j]       (causal: keep u <= t)
    es[u,i]    = sum_j E[h,i,j] s[u,j]
    r[t,i]     = sum_h sum_u Omega[t,u] es[u,i]

Key reassociation: the off-diagonal (full blocks u < bt*128) part factors
through the HEAD-INDEPENDENT Gram matrix C(bt)[j,j'] = sum_{u<bt*128} s[u,j]s[u,j']:
    r_off[t,:] = qs[t,:] @ C(bt) @ E_h^T
so per (h, bt):  M^T[j',t] = C(bt)[j,j'] qsT[j,t]  (+ diag:  s[u,j'] OmaskT[u,t])
                 rT[i,t]  += E_h^T[j',i] M^T[j',t]
C(bt) is computed ONCE (prefix-accumulated in one PSUM bank, snapshot per
block), cutting matmul FLOPs ~20% vs a per-head es/G chain, with N=512 streams
vectorized over all 8 heads. Only diagonal 128x128 Omega^T blocks are
materialized, masked by one DVE multiply per 4-head half.

Profiling findings baked into v12 (graded time = last-matmul-end - ~6.1us
preamble + ~12.1us fixed framework epilogue, so only last-MM-end matters):
  - inputs pre-swizzled host-side to partition-major (contiguous 0.5-4KB
    per-partition runs) and spread across ALL FIVE engine DGE queues with
    small first-needed chunks, so the first real matmul starts ~4us after
    the preamble instead of ~6.7us and phase 1 never stalls on data;
  - 7 warm-up matmuls on a zeroed scratch tile issued while DMAs land keep
    the PE HAM activity window busy so real matmuls run at 2.4GHz from the
    start (v11 ran its whole first phase at 1.2GHz, HAM flipped 12us in);
  - per-ic single-bank rT PSUM tiles so the final pair's drain + output DMA
    overlap its remaining matmuls.

Distribution: data-parallel over batch (8 batches = 8 cores, no collectives).
All matmuls bf16; f32 PSUM accumulation. Output computed transposed; host
unswizzles.

PSUM: 5 transient 1-bank slots + 1 Gram bank + 2 rT banks = 8 banks.
"""

import numpy as np
import ml_dtypes

import concourse.bacc as bacc
import concourse.mybir as mybir
import concourse.tile as tile
from concourse.bass_utils import run_bass_kernel_spmd

B = 8      # batch (== number of cores)
T = 1024   # tokens
NF = 256   # feature dim n
H = 8      # heads
P = 128    # partitions
TB = T // P    # 8 token blocks
JC = NF // P   # 2 feature chunks
NCORES = 8

F32 = mybir.dt.float32
BF16 = mybir.dt.bfloat16
IS_GE = mybir.AluOpType.is_ge


def _emit(tc, nc, s_d, sT_d, Q_d, ET_d, out_d, ctx):
    res = ctx.enter_context(tc.tile_pool(name="res", bufs=1))
    omdp = ctx.enter_context(tc.tile_pool(name="omdp", bufs=2))
    msp = ctx.enter_context(tc.tile_pool(name="msp", bufs=2))
    pap = ctx.enter_context(tc.tile_pool(name="pap", bufs=5, space="PSUM"))
    pcp = ctx.enter_context(tc.tile_pool(name="pcp", bufs=1, space="PSUM"))
    prp = ctx.enter_context(tc.tile_pool(name="prp", bufs=2, space="PSUM"))

    s_sb = res.tile([P, TB, NF], BF16)        # [u%128, uc, j]
    sT_sb = res.tile([P, 2, JC, 512], BF16)   # [j%128, tcx, jc, t%512]
    Q_sb = res.tile([P, H * JC, NF], BF16)    # [i%128, h*2+ic, j]
    ET_sb = res.tile([P, H * JC, NF], BF16)   # [j'%128, h*2+jpc, i]
    qsT = res.tile([P, JC, H, T], BF16)       # [j%128, jc, h, t]
    csnap = res.tile([P, JC, TB - 1, NF], BF16)  # [j%128, jc, k, j'] = Csum(k+1)
    mask4 = res.tile([P, 4, P], BF16)         # [u, 4, t]: 1 where u <= t
    scr = res.tile([P, 512], BF16)            # warm-up scratch
    r_out = res.tile([P, TB // 2, JC, 2 * P], F32)  # [i%128, pair, ic, t%256]

    # Input DMAs: first-needed-first, deadline-balanced over the three DGE
    # queues (sync/scalar HWDGE ~64B/ns from ~1.7us post-preamble; gpsimd
    # SWDGE ~180B/ns but ~3.5us descriptor-gen lead time).
    nc.sync.dma_start(out=sT_sb[:, 0, 0], in_=sT_d[:, 0, 0])
    nc.scalar.dma_start(out=Q_sb[:, 0:4, :], in_=Q_d[:, 0:4, :])
    nc.sync.dma_start(out=Q_sb[:, 4:8, :], in_=Q_d[:, 4:8, :])
    nc.scalar.dma_start(out=Q_sb[:, 8:12, :], in_=Q_d[:, 8:12, :])
    nc.sync.dma_start(out=sT_sb[:, 1], in_=sT_d[:, 1])
    nc.scalar.dma_start(out=Q_sb[:, 12:, :], in_=Q_d[:, 12:, :])
    nc.gpsimd.dma_start(out=sT_sb[:, 0, 1], in_=sT_d[:, 0, 1])
    nc.gpsimd.dma_start(out=s_sb, in_=s_d)
    nc.gpsimd.memset(mask4, 1.0)
    nc.gpsimd.affine_select(
        out=mask4, in_=mask4,
        pattern=[[0, 4], [1, P]],
        compare_op=IS_GE,   # keep 1.0 where t - u >= 0, else 0
        fill=0.0, base=0, channel_multiplier=-1,
    )
    nc.gpsimd.dma_start(out=ET_sb[:, 0:8, :], in_=ET_d[:, 0:8, :])
    nc.sync.dma_start(out=ET_sb[:, 8:, :], in_=ET_d[:, 8:, :])

    # PE warm-up: HAM un-throttles after ~3.4us of sustained activity; burn
    # the input-DMA head on dummy matmuls so real ones start at 2.4GHz.
    nc.vector.memset(scr, 0.0)
    wu = pap.tile([P, 512], F32, tag="pw", name="wu")
    for w in range(10):
        nc.tensor.matmul(
            wu, lhsT=scr[:, 0:P], rhs=scr,
            start=(w == 0), stop=(w == 9), skip_group_check=True)

    movers = [nc.vector.tensor_copy, nc.scalar.copy]
    mv = [0]

    def mover(out, in_):
        movers[mv[0] % 2](out=out, in_=in_)
        mv[0] += 1

    # ---- phase 1: qsT for all heads + Gram prefix chain ----
    Cp = pcp.tile([P, JC, NF], F32, name="Cp")

    def qs_group(h, jc, tcx):
        pw = pap.tile([P, 512], F32, tag="pw", name="pwq")
        for ic in range(JC):
            nc.tensor.matmul(
                pw,
                lhsT=Q_sb[:, h * JC + ic, jc * P:(jc + 1) * P],
                rhs=sT_sb[:, tcx, ic, :],
                start=(ic == 0), stop=(ic == JC - 1),
                skip_group_check=True,
            )
        mover(qsT[:, jc, h, tcx * 512:(tcx + 1) * 512], pw)

    def c_link(uc):
        # Cp += s[uc]^T s[uc]; snapshot Csum(uc+1) for the bt loop
        for jc in range(JC):
            nc.tensor.matmul(
                Cp[:, jc, :],
                lhsT=s_sb[:, uc, jc * P:(jc + 1) * P],
                rhs=s_sb[:, uc, :],
                start=(uc == 0 and jc == 0),
                stop=(uc == TB - 2 and jc == JC - 1),
                skip_group_check=True,
            )
        mover(csnap[:, :, uc, :], Cp)

    qjobs = [(h, jc, tcx) for tcx in range(2) for h in range(H)
             for jc in range(JC)]
    k = 0
    for gi, (h, jc, tcx) in enumerate(qjobs):
        qs_group(h, jc, tcx)
        if gi >= 11 and gi % 2 == 1 and k < TB - 1:
            c_link(k)
            k += 1

    # ---- phase 2: bt loop ----
    def rt_group(pair, ms, last):
        prs = [prp.tile([P, 2 * P], F32, tag="pr", name=f"pr{ic}")
               for ic in range(JC)]
        for ic in range(JC):
            n = 0
            for h in range(H):
                for jpc in range(JC):
                    nc.tensor.matmul(
                        prs[ic],
                        lhsT=ET_sb[:, h * JC + jpc, ic * P:(ic + 1) * P],
                        rhs=ms[:, jpc, h, :],
                        start=(n == 0), stop=(n == H * JC - 1),
                        skip_group_check=True,
                    )
                    n += 1
            # drain + store this half while the other half's matmuls run
            eng = [nc.vector.tensor_copy, nc.scalar.copy][ic]
            eng(out=r_out[:, pair, ic, :], in_=prs[ic])
            q = [nc.sync, nc.scalar][ic] if last else [nc.sync, nc.scalar][pair % 2]
            q.dma_start(out=out_d[:, pair, ic], in_=r_out[:, pair, ic])

    ms_cur = None
    ms_prev = None
    for bt in range(TB):
        pair, sub = divmod(bt, 2)
        if sub == 0:
            ms_cur = msp.tile([P, JC, H, 2 * P], BF16, tag="ms",
                              name=f"ms{pair}")
        # diagonal OmegaT for all 8 heads: OmT[u, h, t] = sum_j s[u,j] qsT[j,t]
        oms = []
        for half in range(2):
            om = pap.tile([P, 4, P], F32, tag="pw", name=f"om{half}")
            for jc in range(JC):
                nc.tensor.matmul(
                    om,
                    lhsT=sT_sb[:, bt // 4, jc,
                               (bt % 4) * P:(bt % 4 + 1) * P],
                    rhs=qsT[:, jc, 4 * half:4 * half + 4, bt * P:(bt + 1) * P],
                    start=(jc == 0), stop=(jc == JC - 1),
                    skip_group_check=True,
                )
            oms.append(om)
        # off-diagonal via Gram: M^T[j', h, t] = Csum(bt)[j,j'] qsT[j, h, t]
        mps = []
        for jpc in range(JC):
            for half in range(2):
                mp = pap.tile([P, 4, P], F32, tag="pw", name=f"mp{jpc}{half}")
                if bt >= 1:
                    for jc in range(JC):
                        nc.tensor.matmul(
                            mp,
                            lhsT=csnap[:, jc, bt - 1, jpc * P:(jpc + 1) * P],
                            rhs=qsT[:, jc, 4 * half:4 * half + 4,
                                    bt * P:(bt + 1) * P],
                            start=(jc == 0), stop=False,
                            skip_group_check=True,
                        )
                mps.append(mp)
        # previous pair's rT matmuls slot in here: keeps the PE busy while
        # this bt's masks run on the DVE, and gives the msnap movers slack
        if bt >= 2 and sub == 0:
            rt_group(pair - 1, ms_prev, last=False)
        # causal mask on the diagonal blocks (keep u <= t)
        omd = omdp.tile([P, H, P], BF16, tag="omd", name="omd")
        nc.vector.tensor_mul(omd[:, 0:4, :], oms[0], mask4)
        nc.vector.tensor_mul(omd[:, 4:8, :], oms[1], mask4)
        # diag contribution: M^T[j', h, t] += s[u, j'] OmaskT[u, h, t]
        for jpc in range(JC):
            for half in range(2):
                nc.tensor.matmul(
                    mps[jpc * 2 + half],
                    lhsT=s_sb[:, bt, jpc * P:(jpc + 1) * P],
                    rhs=omd[:, 4 * half:4 * half + 4, :],
                    start=(bt == 0), stop=True,
                    skip_group_check=True,
                )
        # snap engines 3:1 scalar:vector — vector also owns the masks
        snap_engs = [nc.scalar.copy, nc.vector.tensor_copy,
                     nc.scalar.copy, nc.scalar.copy]
        for jpc in range(JC):
            for half in range(2):
                snap_engs[jpc * 2 + half](
                    out=ms_cur[:, jpc, 4 * half:4 * half + 4,
                               sub * P:(sub + 1) * P],
                    in_=mps[jpc * 2 + half])
        if sub == 1:
            ms_prev = ms_cur
    rt_group(TB // 2 - 1, ms_prev, last=True)


def build():
    from contextlib import ExitStack

    nc = bacc.Bacc(
        "TRN2",
        target_bir_lowering=False,
        debug=False,
        enable_asserts=False,
        num_devices=NCORES,
    )
    s_d = nc.dram_tensor("s", [P, TB, NF], BF16, kind="ExternalInput").ap()
    sT_d = nc.dram_tensor(
        "sT", [P, 2, JC, 512], BF16, kind="ExternalInput").ap()
    Q_d = nc.dram_tensor(
        "Q", [P, H * JC, NF], BF16, kind="ExternalInput").ap()
    ET_d = nc.dram_tensor(
        "ET", [P, H * JC, NF], BF16, kind="ExternalInput").ap()
    out_d = nc.dram_tensor(
        "out", [P, TB // 2, JC, 2 * P], F32, kind="ExternalOutput").ap()
    with tile.TileContext(nc) as tc:
        with ExitStack() as ctx:
            _emit(tc, nc, s_d, sT_d, Q_d, ET_d, out_d, ctx)
    nc.compile()
    return nc


_NC = None


def _get_nc():
    global _NC
    if _NC is None:
        _NC = build()
    return _NC


def _in_maps(s, Q, E):
    bf = ml_dtypes.bfloat16
    s = np.asarray(s, dtype=np.float32)
    Q = np.asarray(Q, dtype=np.float32)
    E = np.asarray(E, dtype=np.float32)
    # partition-major swizzles (one contiguous run per partition per DMA)
    # Q_sw[p, h*2+ic, j] = Q[h, ic*128+p, j]
    Q_sw = np.ascontiguousarray(
        Q.reshape(H, JC, P, NF).transpose(2, 0, 1, 3).reshape(P, H * JC, NF)
    ).astype(bf)
    # ET_sw[p, h*2+jpc, i] = E[h, i, jpc*128+p]
    ET_sw = np.ascontiguousarray(
        E.transpose(0, 2, 1).reshape(H, JC, P, NF).transpose(2, 0, 1, 3)
        .reshape(P, H * JC, NF)).astype(bf)
    maps = []
    for b in range(B):
        sb = s[b]
        # s_sw[p, uc, j] = s[uc*128+p, j]
        s_sw = np.ascontiguousarray(
            sb.reshape(TB, P, NF).transpose(1, 0, 2)).astype(bf)
        # sT_sw[p, tcx, jc, t'] = s[tcx*512+t', jc*128+p]
        sT_sw = np.ascontiguousarray(
            sb.T.reshape(JC, P, 2, 512).transpose(1, 2, 0, 3)).astype(bf)
        maps.append({"s": s_sw, "sT": sT_sw, "Q": Q_sw, "ET": ET_sw})
    return maps


def _unswizzle_out(o):
    # o[p, pair, ic, t'] -> r[pair*256+t', ic*128+p]
    return np.ascontiguousarray(
        o.transpose(1, 3, 2, 0).reshape(T, NF))


def kernel(s, Q, E):
    nc = _get_nc()
    res = run_bass_kernel_spmd(
        nc, _in_maps(s, Q, E), core_ids=list(range(NCORES)))
    return np.stack(
        [_unswizzle_out(res.results[b]["out"]) for b in range(B)], axis=0)


def run_profiled(s, Q, E, tmpdir=None):
    nc = _get_nc()
    res = run_bass_kernel_spmd(
        nc, _in_maps(s, Q, E), core_ids=list(range(NCORES)),
        trace=True, tmpdir=tmpdir)
    out = np.stack(
        [_unswizzle_out(res.results[b]["out"]) for b in range(B)], axis=0)
    return out, res.exec_time_ns
